# revision 1
# baseline (speedup 1.0000x reference)
"""GNN message-passing (SpMM + dense transform) Trainium2 kernel.

out[i] = (sum_{e: row[e]==i} vals[e] * x[col[e]]) @ W + b

Strategy (8 NeuronCores, SPMD single program):
- Host packs nodes into 1600 blocks (<=64 nodes, <=640 edges each) via LPT
  bin-packing; 200 blocks per core; each block = 5 chunks of 128 edge slots.
- Per chunk: indirect-DMA gather of 128 x-rows (one per partition), a DVE
  tensor_scalar builds a vals-weighted one-hot [128, 64] from a constant
  iota, and a fp32 matmul accumulates accT[64 feats, 64 rows] in PSUM.
- Per block: ACT evacuates accT, one matmul with W (outT = W.T @ accT),
  ACT adds bias, DMA out. Host unpermutes rows at the end.
"""
import sys
import heapq

for _p in ("/opt/trn_rl_repo", "/root/.axon_site/_ro/trn_rl_repo"):
    if _p not in sys.path:
        sys.path.append(_p)

import numpy as np

N_NODES = 100000
N_EDGES = 1000000
F = 64
P = 128
W_R = 64          # rows per block
CPB = 5           # chunks per block
EPB = CPB * P     # edge slots per block = 640
NBLK = 1600       # total blocks
NCORE = 8
BPC = NBLK // NCORE   # blocks per core = 200
NCH = BPC * CPB       # chunks per core = 1000

_cache = {}
LAST = {}  # debug/profiling handle: {"nc": ..., "in_maps": [...]}


def _build_program():
    import concourse.bass as bass
    import concourse.bacc as bacc
    import concourse.mybir as mybir
    import concourse.tile as tile

    nc = bacc.Bacc(trn_type="TRN2", dynamic_dma_scratch_size=65536)
    f32 = mybir.dt.float32
    d_x = nc.declare_dram_parameter("x", [N_NODES, F], f32, isOutput=False)
    d_gidx = nc.declare_dram_parameter("gidx", [P, NCH], mybir.dt.int32, isOutput=False)
    d_rl = nc.declare_dram_parameter("rl", [P, NCH], f32, isOutput=False)
    d_vals = nc.declare_dram_parameter("vals", [P, NCH], f32, isOutput=False)
    d_iota = nc.declare_dram_parameter("iota", [P, W_R + 1], f32, isOutput=False)
    d_W = nc.declare_dram_parameter("W", [F, F], f32, isOutput=False)
    d_b = nc.declare_dram_parameter("b", [F, 1], f32, isOutput=False)
    d_out = nc.declare_dram_parameter("out", [BPC, F, W_R], f32, isOutput=True)

    with tile.TileContext(nc) as tc:
        with (
            tc.tile_pool(name="const", bufs=1) as constp,
            tc.tile_pool(name="g", bufs=16) as gp,
            tc.tile_pool(name="oh", bufs=8) as ohp,
            tc.tile_pool(name="ev", bufs=4) as evp,
            tc.tile_pool(name="accp", bufs=2, space="PSUM") as accp,
            tc.tile_pool(name="outp", bufs=2, space="PSUM") as outpp,
        ):
            t_gidx = constp.tile([P, NCH], mybir.dt.int32)
            t_rl = constp.tile([P, NCH], f32)
            t_vals = constp.tile([P, NCH], f32)
            t_iota = constp.tile([P, W_R + 1], f32)
            t_W = constp.tile([F, F], f32)
            t_b = constp.tile([F, 1], f32)
            nc.sync.dma_start(out=t_gidx[:], in_=d_gidx[:])
            nc.sync.dma_start(out=t_rl[:], in_=d_rl[:])
            nc.sync.dma_start(out=t_vals[:], in_=d_vals[:])
            nc.sync.dma_start(out=t_iota[:], in_=d_iota[:])
            nc.sync.dma_start(out=t_W[:], in_=d_W[:])
            nc.sync.dma_start(out=t_b[:], in_=d_b[:])

            for blk in range(BPC):
                t_acc = accp.tile([F, W_R], f32, space="PSUM")
                for ci in range(CPB):
                    c = blk * CPB + ci
                    t_g = gp.tile([P, F], f32)
                    nc.gpsimd.indirect_dma_start(
                        out=t_g[:],
                        out_offset=None,
                        in_=d_x[:],
                        in_offset=bass.IndirectOffsetOnAxis(
                            ap=t_gidx[:, c : c + 1], axis=0
                        ),
                    )
                    t_oh = ohp.tile([P, W_R + 1], f32)
                    nc.vector.tensor_scalar(
                        out=t_oh[:],
                        in0=t_iota[:],
                        scalar1=t_rl[:, c : c + 1],
                        scalar2=t_vals[:, c : c + 1],
                        op0=mybir.AluOpType.is_equal,
                        op1=mybir.AluOpType.mult,
                    )
                    nc.tensor.matmul(
                        out=t_acc[:],
                        lhsT=t_g[:],
                        rhs=t_oh[:, :W_R],
                        start=(ci == 0),
                        stop=(ci == CPB - 1),
                    )
                t_accs = evp.tile([F, W_R], f32)
                nc.scalar.copy(t_accs[:], t_acc[:])
                t_out = outpp.tile([F, W_R], f32, space="PSUM")
                nc.tensor.matmul(
                    out=t_out[:], lhsT=t_W[:], rhs=t_accs[:], start=True, stop=True
                )
                t_outs = evp.tile([F, W_R], f32)
                nc.scalar.add(t_outs[:], t_out[:], t_b[:, :1])
                nc.sync.dma_start(out=d_out[blk], in_=t_outs[:])

    nc.finalize()
    return nc


def _pack(rows):
    """LPT bin-packing of nodes into NBLK blocks (<=W_R nodes, <=EPB edges).

    Returns node_block[n], node_local[n]."""
    deg = np.bincount(rows, minlength=N_NODES)
    order = np.argsort(-deg, kind="stable")
    node_block = np.empty(N_NODES, dtype=np.int64)
    node_local = np.empty(N_NODES, dtype=np.int64)
    heap = [(0, b) for b in range(NBLK)]
    heapq.heapify(heap)
    bin_nodes = np.zeros(NBLK, dtype=np.int64)
    bin_edges = np.zeros(NBLK, dtype=np.int64)
    spill = []
    for n in order:
        d = int(deg[n])
        placed = False
        tmp = []
        while heap:
            e, b = heapq.heappop(heap)
            if e != bin_edges[b] or bin_nodes[b] >= W_R:
                continue  # stale or node-full entry
            if e + d <= EPB:
                node_block[n] = b
                node_local[n] = bin_nodes[b]
                bin_nodes[b] += 1
                bin_edges[b] += d
                if bin_nodes[b] < W_R:
                    heapq.heappush(heap, (int(bin_edges[b]), b))
                placed = True
                break
            else:
                tmp.append((e, b))
        for item in tmp:
            heapq.heappush(heap, item)
        if not placed:
            spill.append(n)
    if spill:
        # first-fit for spilled nodes (rare)
        for n in spill:
            d = int(deg[n])
            cand = np.where((bin_nodes < W_R) & (bin_edges + d <= EPB))[0]
            if len(cand) == 0:
                raise RuntimeError("packing failed")
            b = int(cand[0])
            node_block[n] = b
            node_local[n] = bin_nodes[b]
            bin_nodes[b] += 1
            bin_edges[b] += d
    return node_block, node_local


def kernel(x, adj_vals, adj_row, adj_col, W, b):
    rows = np.asarray(adj_row).astype(np.int64)
    cols = np.asarray(adj_col).astype(np.int64)
    vals = np.asarray(adj_vals).astype(np.float32)
    x = np.ascontiguousarray(np.asarray(x, dtype=np.float32))
    W = np.asarray(W, dtype=np.float32)
    b = np.asarray(b, dtype=np.float32)

    node_block, node_local = _pack(rows)

    # edge -> (block, slot-within-block)
    eb = node_block[rows]
    order = np.argsort(eb, kind="stable")
    eb_sorted = eb[order]
    counts = np.bincount(eb_sorted, minlength=NBLK)
    starts = np.concatenate([[0], np.cumsum(counts)[:-1]])
    pos = np.arange(N_EDGES) - np.repeat(starts, counts)

    core = eb_sorted // BPC
    chunk = (eb_sorted % BPC) * CPB + pos // P
    part = pos % P

    gidx_all = np.zeros((NCORE, P, NCH), dtype=np.int32)
    rl_all = np.zeros((NCORE, P, NCH), dtype=np.float32)
    vals_all = np.zeros((NCORE, P, NCH), dtype=np.float32)
    gidx_all[core, part, chunk] = cols[order].astype(np.int32)
    rl_all[core, part, chunk] = node_local[rows[order]].astype(np.float32)
    vals_all[core, part, chunk] = vals[order]

    iota_np = np.tile(np.arange(W_R + 1, dtype=np.float32), (P, 1)).copy()
    b2 = np.ascontiguousarray(b.reshape(F, 1))

    key = "prog"
    if key not in _cache:
        _cache[key] = _build_program()
    nc = _cache[key]

    from concourse.bass_utils import run_bass_kernel_spmd

    in_maps = []
    for k in range(NCORE):
        in_maps.append(
            {
                "x": x,
                "gidx": np.ascontiguousarray(gidx_all[k]),
                "rl": np.ascontiguousarray(rl_all[k]),
                "vals": np.ascontiguousarray(vals_all[k]),
                "iota": iota_np,
                "W": W,
                "b": b2,
            }
        )
    LAST["nc"] = nc
    LAST["in_maps"] = in_maps
    res = run_bass_kernel_spmd(nc, in_maps, list(range(NCORE)))
    LAST["res"] = res

    out_full = np.zeros((N_NODES, F), dtype=np.float32)
    nodes = np.arange(N_NODES)
    nb = node_block[nodes]
    for k in range(NCORE):
        sel = (nb // BPC) == k
        blk = (nb[sel] % BPC).astype(np.int64)
        r = node_local[nodes[sel]].astype(np.int64)
        big = res.results[k]["out"]  # [BPC, F, W_R]
        out_full[nodes[sel]] = big[blk, :, r]
    return out_full



# revision 4
# speedup vs baseline: 39.9088x; 39.9088x over previous
"""GNN message-passing (SpMM + dense transform) Trainium2 kernel.

out[i] = (sum_{e: row[e]==i} vals[e] * x[col[e]]) @ W + b

Strategy (8 NeuronCores, SPMD single program):
- Host packs nodes into 1600 blocks (<=64 nodes, <=640 edges each) via LPT
  bin-packing; 200 blocks per core; each block = 5 chunks of 128 edge slots.
- x is sharded 1/8 per core (bf16) and AllGathered on device over NeuronLink,
  so the host->device tunnel carries x once instead of 8 replicas.
- Per chunk: indirect-DMA gather of 128 x-rows (one per partition, bf16), a
  DVE tensor_scalar builds a vals-weighted one-hot [128, 64] from a constant
  iota, and a bf16 matmul accumulates accT[64 feats, 64 rows] in fp32 PSUM.
- Per block: ACT evacuates accT (fp32), one fp32 matmul with W
  (outT = W.T @ accT), ACT adds bias and casts to bf16, DMA out.
  Host unpermutes rows at the end.
"""
import sys
import heapq

for _p in ("/opt/trn_rl_repo", "/root/.axon_site/_ro/trn_rl_repo"):
    if _p not in sys.path:
        sys.path.append(_p)

import numpy as np
import ml_dtypes

N_NODES = 100000
N_EDGES = 1000000
F = 64
P = 128
W_R = 64          # rows per block
CPB = 5           # chunks per block
EPB = CPB * P     # edge slots per block = 640
NBLK = 1600       # total blocks
NCORE = 8
BPC = NBLK // NCORE   # blocks per core = 200
NCH = BPC * CPB       # chunks per core = 1000
NSH = N_NODES // NCORE  # x rows per core shard = 12500

_cache = {}
LAST = {}  # debug/profiling handle: {"nc": ..., "in_maps": [...]}


def _build_program():
    import concourse.bass as bass
    import concourse.bacc as bacc
    import concourse.mybir as mybir
    import concourse.tile as tile

    nc = bacc.Bacc(trn_type="TRN2", dynamic_dma_scratch_size=65536)
    f32 = mybir.dt.float32
    bf16 = mybir.dt.bfloat16
    d_xs = nc.declare_dram_parameter("xs", [NSH, F], bf16, isOutput=False)
    d_gidx = nc.declare_dram_parameter("gidx", [P, NCH], mybir.dt.int32, isOutput=False)
    d_rl = nc.declare_dram_parameter("rl", [P, NCH], f32, isOutput=False)
    d_vals = nc.declare_dram_parameter("vals", [P, NCH], f32, isOutput=False)
    d_iota = nc.declare_dram_parameter("iota", [P, W_R + 1], bf16, isOutput=False)
    d_W = nc.declare_dram_parameter("W", [F, F], f32, isOutput=False)
    d_b = nc.declare_dram_parameter("b", [F, 1], f32, isOutput=False)
    d_out = nc.declare_dram_parameter("out", [BPC, F, W_R], bf16, isOutput=True)

    with tile.TileContext(nc) as tc:
        # Collectives can't touch I/O tensors directly: bounce the local x
        # shard into internal DRAM, AllGather to a full copy per core.
        xs_bounce, _ = tc.tile([NSH, F], bf16, space="DRAM", name="xs_bounce")
        xfull, _ = tc.tile(
            [N_NODES, F], bf16, space="DRAM", addr_space="Shared", name="xfull"
        )
        nc.gpsimd.dma_start(out=xs_bounce[:], in_=d_xs[:])
        nc.gpsimd.collective_compute(
            "AllGather",
            mybir.AluOpType.bypass,
            replica_groups=[list(range(NCORE))],
            ins=[xs_bounce.opt()],
            outs=[xfull.opt()],
        )
        with (
            tc.tile_pool(name="const", bufs=1) as constp,
            tc.tile_pool(name="g", bufs=16) as gp,
            tc.tile_pool(name="oh", bufs=8) as ohp,
            tc.tile_pool(name="ev", bufs=4) as evp,
            tc.tile_pool(name="accp", bufs=2, space="PSUM") as accp,
            tc.tile_pool(name="outp", bufs=2, space="PSUM") as outpp,
        ):
            t_gidx = constp.tile([P, NCH], mybir.dt.int32)
            t_rl = constp.tile([P, NCH], f32)
            t_vals = constp.tile([P, NCH], f32)
            t_iota = constp.tile([P, W_R + 1], bf16)
            t_W = constp.tile([F, F], f32)
            t_b = constp.tile([F, 1], f32)
            nc.sync.dma_start(out=t_gidx[:], in_=d_gidx[:])
            nc.sync.dma_start(out=t_rl[:], in_=d_rl[:])
            nc.sync.dma_start(out=t_vals[:], in_=d_vals[:])
            nc.sync.dma_start(out=t_iota[:], in_=d_iota[:])
            nc.sync.dma_start(out=t_W[:], in_=d_W[:])
            nc.sync.dma_start(out=t_b[:], in_=d_b[:])

            for blk in range(BPC):
                t_acc = accp.tile([F, W_R], f32, space="PSUM")
                for ci in range(CPB):
                    c = blk * CPB + ci
                    t_g = gp.tile([P, F], bf16)
                    nc.gpsimd.indirect_dma_start(
                        out=t_g[:],
                        out_offset=None,
                        in_=xfull[:],
                        in_offset=bass.IndirectOffsetOnAxis(
                            ap=t_gidx[:, c : c + 1], axis=0
                        ),
                    )
                    t_oh = ohp.tile([P, W_R + 1], bf16)
                    nc.vector.tensor_scalar(
                        out=t_oh[:],
                        in0=t_iota[:],
                        scalar1=t_rl[:, c : c + 1],
                        scalar2=t_vals[:, c : c + 1],
                        op0=mybir.AluOpType.is_equal,
                        op1=mybir.AluOpType.mult,
                    )
                    nc.tensor.matmul(
                        out=t_acc[:],
                        lhsT=t_g[:],
                        rhs=t_oh[:, :W_R],
                        start=(ci == 0),
                        stop=(ci == CPB - 1),
                    )
                t_accs = evp.tile([F, W_R], f32)
                nc.scalar.copy(t_accs[:], t_acc[:])
                t_out = outpp.tile([F, W_R], f32, space="PSUM")
                nc.tensor.matmul(
                    out=t_out[:], lhsT=t_W[:], rhs=t_accs[:], start=True, stop=True
                )
                t_outs = evp.tile([F, W_R], bf16)
                nc.scalar.add(t_outs[:], t_out[:], t_b[:, :1])
                nc.sync.dma_start(out=d_out[blk], in_=t_outs[:])

    nc.finalize()
    return nc


def _pack(rows):
    """LPT bin-packing of nodes into NBLK blocks (<=W_R nodes, <=EPB edges).

    Returns node_block[n], node_local[n]."""
    deg = np.bincount(rows, minlength=N_NODES)
    order = np.argsort(-deg, kind="stable")
    node_block = np.empty(N_NODES, dtype=np.int64)
    node_local = np.empty(N_NODES, dtype=np.int64)
    heap = [(0, b) for b in range(NBLK)]
    heapq.heapify(heap)
    bin_nodes = np.zeros(NBLK, dtype=np.int64)
    bin_edges = np.zeros(NBLK, dtype=np.int64)
    spill = []
    for n in order:
        d = int(deg[n])
        placed = False
        tmp = []
        while heap:
            e, b = heapq.heappop(heap)
            if e != bin_edges[b] or bin_nodes[b] >= W_R:
                continue  # stale or node-full entry
            if e + d <= EPB:
                node_block[n] = b
                node_local[n] = bin_nodes[b]
                bin_nodes[b] += 1
                bin_edges[b] += d
                if bin_nodes[b] < W_R:
                    heapq.heappush(heap, (int(bin_edges[b]), b))
                placed = True
                break
            else:
                tmp.append((e, b))
        for item in tmp:
            heapq.heappush(heap, item)
        if not placed:
            spill.append(n)
    if spill:
        # first-fit for spilled nodes (rare)
        for n in spill:
            d = int(deg[n])
            cand = np.where((bin_nodes < W_R) & (bin_edges + d <= EPB))[0]
            if len(cand) == 0:
                raise RuntimeError("packing failed")
            b = int(cand[0])
            node_block[n] = b
            node_local[n] = bin_nodes[b]
            bin_nodes[b] += 1
            bin_edges[b] += d
    return node_block, node_local


def kernel(x, adj_vals, adj_row, adj_col, W, b):
    rows = np.asarray(adj_row).astype(np.int64)
    cols = np.asarray(adj_col).astype(np.int64)
    vals = np.asarray(adj_vals).astype(np.float32)
    x = np.ascontiguousarray(np.asarray(x, dtype=np.float32))
    W = np.asarray(W, dtype=np.float32)
    b = np.asarray(b, dtype=np.float32)

    node_block, node_local = _pack(rows)

    # edge -> (block, slot-within-block)
    eb = node_block[rows]
    order = np.argsort(eb, kind="stable")
    eb_sorted = eb[order]
    counts = np.bincount(eb_sorted, minlength=NBLK)
    starts = np.concatenate([[0], np.cumsum(counts)[:-1]])
    pos = np.arange(N_EDGES) - np.repeat(starts, counts)

    core = eb_sorted // BPC
    chunk = (eb_sorted % BPC) * CPB + pos // P
    part = pos % P

    gidx_all = np.zeros((NCORE, P, NCH), dtype=np.int32)
    rl_all = np.zeros((NCORE, P, NCH), dtype=np.float32)
    vals_all = np.zeros((NCORE, P, NCH), dtype=np.float32)
    gidx_all[core, part, chunk] = cols[order].astype(np.int32)
    rl_all[core, part, chunk] = node_local[rows[order]].astype(np.float32)
    vals_all[core, part, chunk] = vals[order]

    bf16 = ml_dtypes.bfloat16
    x_bf = x.astype(bf16)
    iota_np = np.tile(np.arange(W_R + 1, dtype=np.float32), (P, 1)).astype(bf16)
    b2 = np.ascontiguousarray(b.reshape(F, 1))

    key = "prog"
    if key not in _cache:
        _cache[key] = _build_program()
    nc = _cache[key]

    from concourse.bass_utils import run_bass_kernel_spmd

    in_maps = []
    for k in range(NCORE):
        in_maps.append(
            {
                "xs": np.ascontiguousarray(x_bf[k * NSH : (k + 1) * NSH]),
                "gidx": np.ascontiguousarray(gidx_all[k]),
                "rl": np.ascontiguousarray(rl_all[k]),
                "vals": np.ascontiguousarray(vals_all[k]),
                "iota": iota_np,
                "W": W,
                "b": b2,
            }
        )
    LAST["nc"] = nc
    LAST["in_maps"] = in_maps
    res = run_bass_kernel_spmd(nc, in_maps, list(range(NCORE)))
    LAST["res"] = res

    out_full = np.zeros((N_NODES, F), dtype=np.float32)
    nodes = np.arange(N_NODES)
    nb = node_block[nodes]
    for k in range(NCORE):
        sel = (nb // BPC) == k
        blk = (nb[sel] % BPC).astype(np.int64)
        r = node_local[nodes[sel]].astype(np.int64)
        big = res.results[k]["out"].astype(np.float32)  # [BPC, F, W_R]
        out_full[nodes[sel]] = big[blk, :, r]
    return out_full


# revision 9
# speedup vs baseline: 139.8444x; 3.5041x over previous
"""GNN message-passing (SpMM + dense transform) Trainium2 kernel.

out[i] = (sum_{e: row[e]==i} vals[e] * x[col[e]]) @ W + b

Strategy (8 NeuronCores, SPMD single program):
- Host packs nodes into 1600 blocks (<=64 nodes, <=640 edges each) via LPT
  bin-packing; 200 blocks per core; each block = 5 chunks of 128 edge slots.
- x is sharded 1/8 per core (bf16) and AllGathered on device over NeuronLink,
  so the host->device tunnel carries x once instead of 8 replicas.
- Per chunk: indirect-DMA gather of 128 x-rows (one per partition, bf16), a
  DVE tensor_scalar builds a vals-weighted one-hot [128, 64] from a constant
  iota, and a bf16 matmul accumulates accT[64 feats, 64 rows] in fp32 PSUM.
- Per block: ACT evacuates accT (fp32), one fp32 matmul with W
  (outT = W.T @ accT), ACT adds bias and casts to bf16, DMA out.
  Host unpermutes rows at the end.
"""
import sys
import heapq

for _p in ("/opt/trn_rl_repo", "/root/.axon_site/_ro/trn_rl_repo"):
    if _p not in sys.path:
        sys.path.append(_p)

import numpy as np
import ml_dtypes

N_NODES = 100000
N_EDGES = 1000000
F = 64
P = 128
W_R = 64          # rows per block
CPB = 5           # chunks per block
EPB = CPB * P     # edge slots per block = 640
NBLK = 1600       # total blocks
NCORE = 8
BPC = NBLK // NCORE   # blocks per core = 200
NCH = BPC * CPB       # chunks per core = 1000
NSH = N_NODES // NCORE  # x rows per core shard = 12500

_cache = {}
LAST = {}  # debug/profiling handle: {"nc": ..., "in_maps": [...]}


def _build_program():
    import concourse.bass as bass
    import concourse.bacc as bacc
    import concourse.mybir as mybir
    import concourse.tile as tile

    nc = bacc.Bacc(trn_type="TRN2", dynamic_dma_scratch_size=65536)
    f32 = mybir.dt.float32
    bf16 = mybir.dt.bfloat16
    i32 = mybir.dt.int32
    d_xs = nc.declare_dram_parameter("xs", [NSH, F], bf16, isOutput=False)
    # gidx packs the one-hot row-local index in bits 24..29 and the gather
    # node index in bits 0..23; unpacked once on device.
    d_gidx = nc.declare_dram_parameter("gidx", [P, NCH], i32, isOutput=False)
    d_vals = nc.declare_dram_parameter("vals", [P, NCH], bf16, isOutput=False)
    d_iota = nc.declare_dram_parameter("iota", [P, W_R + 1], bf16, isOutput=False)
    d_W = nc.declare_dram_parameter("W", [F, F], f32, isOutput=False)
    d_b = nc.declare_dram_parameter("b", [F, 1], f32, isOutput=False)
    d_out = nc.declare_dram_parameter("out", [BPC, F, W_R], bf16, isOutput=True)

    # Collectives can't touch I/O tensors directly: bounce the local x
    # shard into internal DRAM, AllGather to a full copy per core.
    xs_bounce = nc.dram_tensor("xs_bounce", [NSH, F], bf16)
    xfull = nc.dram_tensor("xfull", [N_NODES, F], bf16, addr_space="Shared")

    with tile.TileContext(nc) as tc:
        nc.gpsimd.dma_start(out=xs_bounce[:], in_=d_xs[:])
        nc.gpsimd.collective_compute(
            "AllGather",
            mybir.AluOpType.bypass,
            replica_groups=[list(range(NCORE))],
            ins=[xs_bounce[:].opt()],
            outs=[xfull[:].opt()],
        )
        with (
            tc.tile_pool(name="const", bufs=1) as constp,
            tc.tile_pool(name="g", bufs=8) as gp,
            tc.tile_pool(name="oh", bufs=8) as ohp,
            tc.tile_pool(name="ev", bufs=4) as evp,
            tc.tile_pool(name="accp", bufs=2, space="PSUM") as accp,
            tc.tile_pool(name="outp", bufs=2, space="PSUM") as outpp,
        ):
            t_pk = constp.tile([P, NCH], i32)
            t_gidx = constp.tile([P, NCH], i32)
            t_rli = constp.tile([P, NCH], i32)
            t_rl = constp.tile([P, NCH], f32)
            t_valsb = constp.tile([P, NCH], bf16)
            t_vals = constp.tile([P, NCH], f32)
            t_iota = constp.tile([P, W_R + 1], bf16)
            t_W = constp.tile([F, F], f32)
            t_b = constp.tile([F, 1], f32)
            nc.sync.dma_start(out=t_pk[:], in_=d_gidx[:])
            nc.sync.dma_start(out=t_valsb[:], in_=d_vals[:])
            nc.sync.dma_start(out=t_iota[:], in_=d_iota[:])
            nc.sync.dma_start(out=t_W[:], in_=d_W[:])
            nc.sync.dma_start(out=t_b[:], in_=d_b[:])
            nc.vector.tensor_scalar(
                out=t_rli[:], in0=t_pk[:], scalar1=24, scalar2=None,
                op0=mybir.AluOpType.logical_shift_right,
            )
            nc.vector.tensor_copy(t_rl[:], t_rli[:])
            nc.vector.tensor_scalar(
                out=t_gidx[:], in0=t_pk[:], scalar1=0x00FFFFFF, scalar2=None,
                op0=mybir.AluOpType.bitwise_and,
            )
            nc.vector.tensor_copy(t_vals[:], t_valsb[:])

            for blk in range(BPC):
                t_acc = accp.tile([F, W_R], f32, space="PSUM")
                for ci in range(CPB):
                    c = blk * CPB + ci
                    t_g = gp.tile([P, F], bf16)
                    nc.gpsimd.indirect_dma_start(
                        out=t_g[:],
                        out_offset=None,
                        in_=xfull[:],
                        in_offset=bass.IndirectOffsetOnAxis(
                            ap=t_gidx[:, c : c + 1], axis=0
                        ),
                    )
                    t_oh = ohp.tile([P, W_R + 1], bf16)
                    nc.vector.tensor_scalar(
                        out=t_oh[:],
                        in0=t_iota[:],
                        scalar1=t_rl[:, c : c + 1],
                        scalar2=t_vals[:, c : c + 1],
                        op0=mybir.AluOpType.is_equal,
                        op1=mybir.AluOpType.mult,
                    )
                    nc.tensor.matmul(
                        out=t_acc[:],
                        lhsT=t_g[:],
                        rhs=t_oh[:, :W_R],
                        start=(ci == 0),
                        stop=(ci == CPB - 1),
                    )
                t_accs = evp.tile([F, W_R], f32)
                nc.scalar.copy(t_accs[:], t_acc[:])
                t_out = outpp.tile([F, W_R], f32, space="PSUM")
                nc.tensor.matmul(
                    out=t_out[:], lhsT=t_W[:], rhs=t_accs[:], start=True, stop=True
                )
                t_outs = evp.tile([F, W_R], bf16)
                nc.scalar.add(t_outs[:], t_out[:], t_b[:, :1])
                nc.sync.dma_start(out=d_out[blk], in_=t_outs[:])

    nc.finalize()
    return nc


def _pack(rows):
    """LPT bin-packing of nodes into NBLK blocks (<=W_R nodes, <=EPB edges).

    Returns node_block[n], node_local[n]."""
    deg = np.bincount(rows, minlength=N_NODES)
    order = np.argsort(-deg, kind="stable")
    node_block = np.empty(N_NODES, dtype=np.int64)
    node_local = np.empty(N_NODES, dtype=np.int64)
    heap = [(0, b) for b in range(NBLK)]
    heapq.heapify(heap)
    bin_nodes = np.zeros(NBLK, dtype=np.int64)
    bin_edges = np.zeros(NBLK, dtype=np.int64)
    spill = []
    for n in order:
        d = int(deg[n])
        placed = False
        tmp = []
        while heap:
            e, b = heapq.heappop(heap)
            if e != bin_edges[b] or bin_nodes[b] >= W_R:
                continue  # stale or node-full entry
            if e + d <= EPB:
                node_block[n] = b
                node_local[n] = bin_nodes[b]
                bin_nodes[b] += 1
                bin_edges[b] += d
                if bin_nodes[b] < W_R:
                    heapq.heappush(heap, (int(bin_edges[b]), b))
                placed = True
                break
            else:
                tmp.append((e, b))
        for item in tmp:
            heapq.heappush(heap, item)
        if not placed:
            spill.append(n)
    if spill:
        # first-fit for spilled nodes (rare)
        for n in spill:
            d = int(deg[n])
            cand = np.where((bin_nodes < W_R) & (bin_edges + d <= EPB))[0]
            if len(cand) == 0:
                raise RuntimeError("packing failed")
            b = int(cand[0])
            node_block[n] = b
            node_local[n] = bin_nodes[b]
            bin_nodes[b] += 1
            bin_edges[b] += d
    return node_block, node_local


def kernel(x, adj_vals, adj_row, adj_col, W, b):
    rows = np.asarray(adj_row).astype(np.int64)
    cols = np.asarray(adj_col).astype(np.int64)
    vals = np.asarray(adj_vals).astype(np.float32)
    x = np.ascontiguousarray(np.asarray(x, dtype=np.float32))
    W = np.asarray(W, dtype=np.float32)
    b = np.asarray(b, dtype=np.float32)

    node_block, node_local = _pack(rows)

    # edge -> (block, slot-within-block)
    eb = node_block[rows]
    order = np.argsort(eb, kind="stable")
    eb_sorted = eb[order]
    counts = np.bincount(eb_sorted, minlength=NBLK)
    starts = np.concatenate([[0], np.cumsum(counts)[:-1]])
    pos = np.arange(N_EDGES) - np.repeat(starts, counts)

    core = eb_sorted // BPC
    chunk = (eb_sorted % BPC) * CPB + pos // P
    part = pos % P

    gidx_all = np.zeros((NCORE, P, NCH), dtype=np.int32)
    vals_all = np.zeros((NCORE, P, NCH), dtype=np.float32)
    packed = (cols[order] | (node_local[rows[order]] << 24)).astype(np.int32)
    gidx_all[core, part, chunk] = packed
    vals_all[core, part, chunk] = vals[order]

    bf16 = ml_dtypes.bfloat16
    x_bf = x.astype(bf16)
    vals_bf = vals_all.astype(bf16)
    iota_np = np.tile(np.arange(W_R + 1, dtype=np.float32), (P, 1)).astype(bf16)
    b2 = np.ascontiguousarray(b.reshape(F, 1))

    key = "prog"
    if key not in _cache:
        _cache[key] = _build_program()
    nc = _cache[key]

    from concourse.bass_utils import run_bass_kernel_spmd

    in_maps = []
    for k in range(NCORE):
        in_maps.append(
            {
                "xs": np.ascontiguousarray(x_bf[k * NSH : (k + 1) * NSH]),
                "gidx": np.ascontiguousarray(gidx_all[k]),
                "vals": np.ascontiguousarray(vals_bf[k]),
                "iota": iota_np,
                "W": W,
                "b": b2,
            }
        )
    LAST["nc"] = nc
    LAST["in_maps"] = in_maps
    res = run_bass_kernel_spmd(nc, in_maps, list(range(NCORE)))
    LAST["res"] = res

    out_full = np.zeros((N_NODES, F), dtype=np.float32)
    nodes = np.arange(N_NODES)
    nb = node_block[nodes]
    for k in range(NCORE):
        sel = (nb // BPC) == k
        blk = (nb[sel] % BPC).astype(np.int64)
        r = node_local[nodes[sel]].astype(np.int64)
        big = res.results[k]["out"].astype(np.float32)  # [BPC, F, W_R]
        out_full[nodes[sel]] = big[blk, :, r]
    return out_full


# revision 10
# speedup vs baseline: 242.4543x; 1.7337x over previous
"""GNN message-passing (SpMM + dense transform) Trainium2 kernel.

out[i] = (sum_{e: row[e]==i} vals[e] * x[col[e]]) @ W + b

Strategy (8 NeuronCores, SPMD single program):
- Host packs nodes into 1600 blocks (<=64 nodes, <=640 edges each) via LPT
  bin-packing; 200 blocks per core; each block = 5 chunks of 128 edge slots.
- x is sharded 1/8 per core (bf16) and AllGathered on device over NeuronLink,
  so the host->device tunnel carries x once instead of 8 replicas.
- Edge metadata is one int32 per edge (node index in bits 0..23, one-hot
  row-local index in bits 24..29, unpacked once on device by DVE) plus a
  bf16 val (upcast to fp32 on device) -- 6B/edge over the tunnel.
- Per chunk: indirect-DMA gather of 128 x-rows (one per partition, bf16), a
  DVE tensor_scalar builds a vals-weighted one-hot [128, 64] from a constant
  iota, and a bf16 matmul accumulates accT[64 feats, 64 rows] in fp32 PSUM.
- Per block: ACT evacuates accT (fp32), one fp32 matmul with W
  (outT = W.T @ accT), ACT adds bias and casts to bf16, DMA out.
  Host unpermutes rows at the end.
"""
import sys
import heapq

for _p in ("/opt/trn_rl_repo", "/root/.axon_site/_ro/trn_rl_repo"):
    if _p not in sys.path:
        sys.path.append(_p)

import numpy as np
import ml_dtypes

N_NODES = 100000
N_EDGES = 1000000
F = 64
P = 128
W_R = 64          # rows per block
CPB = 5           # chunks per block
EPB = CPB * P     # edge slots per block = 640
NBLK = 1600       # total blocks
NCORE = 8
BPC = NBLK // NCORE   # blocks per core = 200
NCH = BPC * CPB       # chunks per core = 1000
NSH = N_NODES // NCORE  # x rows per core shard = 12500

_cache = {}
LAST = {}  # debug/profiling handle: {"nc": ..., "in_maps": [...]}


def _build_program():
    import concourse.bass as bass
    import concourse.bacc as bacc
    import concourse.mybir as mybir
    import concourse.tile as tile

    nc = bacc.Bacc(trn_type="TRN2", dynamic_dma_scratch_size=65536)
    f32 = mybir.dt.float32
    bf16 = mybir.dt.bfloat16
    i32 = mybir.dt.int32
    d_xs = nc.declare_dram_parameter("xs", [NSH, F], bf16, isOutput=False)
    # gidx packs the one-hot row-local index in bits 24..29 and the gather
    # node index in bits 0..23; unpacked once on device.
    d_gidx = nc.declare_dram_parameter("gidx", [P, NCH], i32, isOutput=False)
    d_vals = nc.declare_dram_parameter("vals", [P, NCH], bf16, isOutput=False)
    d_iota = nc.declare_dram_parameter("iota", [P, W_R + 1], bf16, isOutput=False)
    d_W = nc.declare_dram_parameter("W", [F, F], f32, isOutput=False)
    d_b = nc.declare_dram_parameter("b", [F, 1], f32, isOutput=False)
    d_out = nc.declare_dram_parameter("out", [BPC, F, W_R], bf16, isOutput=True)

    # Collectives can't touch I/O tensors directly: bounce the local x
    # shard into internal DRAM, AllGather to a full copy per core.
    xs_bounce = nc.dram_tensor("xs_bounce", [NSH, F], bf16)
    xfull = nc.dram_tensor("xfull", [N_NODES, F], bf16, addr_space="Shared")

    with tile.TileContext(nc) as tc:
        nc.gpsimd.dma_start(out=xs_bounce[:], in_=d_xs[:])
        nc.gpsimd.collective_compute(
            "AllGather",
            mybir.AluOpType.bypass,
            replica_groups=[list(range(NCORE))],
            ins=[xs_bounce[:].opt()],
            outs=[xfull[:].opt()],
        )
        with (
            tc.tile_pool(name="const", bufs=1) as constp,
            tc.tile_pool(name="g", bufs=8) as gp,
            tc.tile_pool(name="oh", bufs=8) as ohp,
            tc.tile_pool(name="ev", bufs=4) as evp,
            tc.tile_pool(name="accp", bufs=2, space="PSUM") as accp,
            tc.tile_pool(name="outp", bufs=2, space="PSUM") as outpp,
        ):
            t_pk = constp.tile([P, NCH], i32)
            t_gidx = constp.tile([P, NCH], i32)
            t_rli = constp.tile([P, NCH], i32)
            t_rl = constp.tile([P, NCH], f32)
            t_valsb = constp.tile([P, NCH], bf16)
            t_vals = constp.tile([P, NCH], f32)
            t_iota = constp.tile([P, W_R + 1], bf16)
            t_W = constp.tile([F, F], f32)
            t_b = constp.tile([F, 1], f32)
            nc.sync.dma_start(out=t_pk[:], in_=d_gidx[:])
            nc.sync.dma_start(out=t_valsb[:], in_=d_vals[:])
            nc.sync.dma_start(out=t_iota[:], in_=d_iota[:])
            nc.sync.dma_start(out=t_W[:], in_=d_W[:])
            nc.sync.dma_start(out=t_b[:], in_=d_b[:])
            nc.vector.tensor_scalar(
                out=t_rli[:], in0=t_pk[:], scalar1=24, scalar2=None,
                op0=mybir.AluOpType.logical_shift_right,
            )
            nc.vector.tensor_copy(t_rl[:], t_rli[:])
            nc.vector.tensor_scalar(
                out=t_gidx[:], in0=t_pk[:], scalar1=0x00FFFFFF, scalar2=None,
                op0=mybir.AluOpType.bitwise_and,
            )
            nc.vector.tensor_copy(t_vals[:], t_valsb[:])

            for blk in range(BPC):
                t_acc = accp.tile([F, W_R], f32, space="PSUM")
                for ci in range(CPB):
                    c = blk * CPB + ci
                    t_g = gp.tile([P, F], bf16)
                    nc.gpsimd.indirect_dma_start(
                        out=t_g[:],
                        out_offset=None,
                        in_=xfull[:],
                        in_offset=bass.IndirectOffsetOnAxis(
                            ap=t_gidx[:, c : c + 1], axis=0
                        ),
                    )
                    t_oh = ohp.tile([P, W_R + 1], bf16)
                    nc.vector.tensor_scalar(
                        out=t_oh[:],
                        in0=t_iota[:],
                        scalar1=t_rl[:, c : c + 1],
                        scalar2=t_vals[:, c : c + 1],
                        op0=mybir.AluOpType.is_equal,
                        op1=mybir.AluOpType.mult,
                    )
                    nc.tensor.matmul(
                        out=t_acc[:],
                        lhsT=t_g[:],
                        rhs=t_oh[:, :W_R],
                        start=(ci == 0),
                        stop=(ci == CPB - 1),
                    )
                t_accs = evp.tile([F, W_R], f32)
                nc.scalar.copy(t_accs[:], t_acc[:])
                t_out = outpp.tile([F, W_R], f32, space="PSUM")
                nc.tensor.matmul(
                    out=t_out[:], lhsT=t_W[:], rhs=t_accs[:], start=True, stop=True
                )
                t_outs = evp.tile([F, W_R], bf16)
                nc.scalar.add(t_outs[:], t_out[:], t_b[:, :1])
                nc.sync.dma_start(out=d_out[blk], in_=t_outs[:])

    nc.finalize()
    return nc


def _pack(rows):
    """LPT bin-packing of nodes into NBLK blocks (<=W_R nodes, <=EPB edges).

    Returns node_block[n], node_local[n]."""
    deg = np.bincount(rows, minlength=N_NODES)
    order = np.argsort(-deg, kind="stable")
    node_block = np.empty(N_NODES, dtype=np.int64)
    node_local = np.empty(N_NODES, dtype=np.int64)
    heap = [(0, b) for b in range(NBLK)]
    heapq.heapify(heap)
    bin_nodes = np.zeros(NBLK, dtype=np.int64)
    bin_edges = np.zeros(NBLK, dtype=np.int64)
    spill = []
    for n in order:
        d = int(deg[n])
        placed = False
        tmp = []
        while heap:
            e, b = heapq.heappop(heap)
            if e != bin_edges[b] or bin_nodes[b] >= W_R:
                continue  # stale or node-full entry
            if e + d <= EPB:
                node_block[n] = b
                node_local[n] = bin_nodes[b]
                bin_nodes[b] += 1
                bin_edges[b] += d
                if bin_nodes[b] < W_R:
                    heapq.heappush(heap, (int(bin_edges[b]), b))
                placed = True
                break
            else:
                tmp.append((e, b))
        for item in tmp:
            heapq.heappush(heap, item)
        if not placed:
            spill.append(n)
    if spill:
        # first-fit for spilled nodes (rare)
        for n in spill:
            d = int(deg[n])
            cand = np.where((bin_nodes < W_R) & (bin_edges + d <= EPB))[0]
            if len(cand) == 0:
                raise RuntimeError("packing failed")
            b = int(cand[0])
            node_block[n] = b
            node_local[n] = bin_nodes[b]
            bin_nodes[b] += 1
            bin_edges[b] += d
    return node_block, node_local


def kernel(x, adj_vals, adj_row, adj_col, W, b):
    rows = np.asarray(adj_row).astype(np.int64)
    cols = np.asarray(adj_col).astype(np.int64)
    vals = np.asarray(adj_vals).astype(np.float32)
    x = np.ascontiguousarray(np.asarray(x, dtype=np.float32))
    W = np.asarray(W, dtype=np.float32)
    b = np.asarray(b, dtype=np.float32)

    node_block, node_local = _pack(rows)

    # edge -> (block, slot-within-block)
    eb = node_block[rows]
    order = np.argsort(eb, kind="stable")
    eb_sorted = eb[order]
    counts = np.bincount(eb_sorted, minlength=NBLK)
    starts = np.concatenate([[0], np.cumsum(counts)[:-1]])
    pos = np.arange(N_EDGES) - np.repeat(starts, counts)

    core = eb_sorted // BPC
    chunk = (eb_sorted % BPC) * CPB + pos // P
    part = pos % P

    gidx_all = np.zeros((NCORE, P, NCH), dtype=np.int32)
    vals_all = np.zeros((NCORE, P, NCH), dtype=np.float32)
    packed = (cols[order] | (node_local[rows[order]] << 24)).astype(np.int32)
    gidx_all[core, part, chunk] = packed
    vals_all[core, part, chunk] = vals[order]

    bf16 = ml_dtypes.bfloat16
    x_bf = x.astype(bf16)
    vals_bf = vals_all.astype(bf16)
    iota_np = np.tile(np.arange(W_R + 1, dtype=np.float32), (P, 1)).astype(bf16)
    b2 = np.ascontiguousarray(b.reshape(F, 1))

    key = "prog"
    if key not in _cache:
        _cache[key] = _build_program()
    nc = _cache[key]

    from concourse.bass_utils import run_bass_kernel_spmd

    in_maps = []
    for k in range(NCORE):
        in_maps.append(
            {
                "xs": np.ascontiguousarray(x_bf[k * NSH : (k + 1) * NSH]),
                "gidx": np.ascontiguousarray(gidx_all[k]),
                "vals": np.ascontiguousarray(vals_bf[k]),
                "iota": iota_np,
                "W": W,
                "b": b2,
            }
        )
    LAST["nc"] = nc
    LAST["in_maps"] = in_maps
    res = run_bass_kernel_spmd(nc, in_maps, list(range(NCORE)))
    LAST["res"] = res

    out_full = np.zeros((N_NODES, F), dtype=np.float32)
    nodes = np.arange(N_NODES)
    nb = node_block[nodes]
    for k in range(NCORE):
        sel = (nb // BPC) == k
        blk = (nb[sel] % BPC).astype(np.int64)
        r = node_local[nodes[sel]].astype(np.int64)
        big = res.results[k]["out"].astype(np.float32)  # [BPC, F, W_R]
        out_full[nodes[sel]] = big[blk, :, r]
    return out_full


# revision 16
# speedup vs baseline: 486.5999x; 2.0070x over previous
"""GNN message-passing (SpMM + dense transform) Trainium2 kernel.

out[i] = (sum_{e: row[e]==i} vals[e] * x[col[e]]) @ W + b

Strategy (8 NeuronCores, SPMD single program):
- Host packs nodes into 1600 blocks (<=64 nodes, <=640 edges each) via LPT
  bin-packing; 200 blocks per core; each block = 5 chunks of 128 edge slots.
- x is sharded 1/8 per core (bf16) and AllGathered on device over NeuronLink,
  so the host->device tunnel carries x once instead of 8 replicas.
- Edge metadata is one int32 per edge (node index in bits 0..23, one-hot
  row-local index in bits 24..29, unpacked once on device by DVE) plus a
  bf16 val (upcast to fp32 on device) -- 6B/edge over the tunnel.
- Per chunk: indirect-DMA gather of 128 x-rows (one per partition, bf16), a
  DVE tensor_scalar builds a vals-weighted one-hot [128, 64] from a constant
  iota, and a bf16 matmul accumulates accT[64 feats, 64 rows] in fp32 PSUM.
- Per block: ACT evacuates accT (fp32), one fp32 matmul with W
  (outT = W.T @ accT), ACT adds bias and casts to bf16, DMA out.
  Host unpermutes rows at the end.
"""
import sys
import heapq

for _p in ("/opt/trn_rl_repo", "/root/.axon_site/_ro/trn_rl_repo"):
    if _p not in sys.path:
        sys.path.append(_p)

import numpy as np
import ml_dtypes

N_NODES = 100000
N_EDGES = 1000000
F = 64
P = 128
W_R = 64          # rows per block
CPB = 5           # chunks per block
EPB = CPB * P     # edge slots per block = 640
NBLK = 1600       # total blocks
NCORE = 8
BPC = NBLK // NCORE   # blocks per core = 200
NCH = BPC * CPB       # chunks per core = 1000
NSH = N_NODES // NCORE  # x rows per core shard = 12500
BLOB_COLS = NCH + 500 + 33 + 64 + 1  # metadata blob width in i32 columns

_cache = {}
LAST = {}  # debug/profiling handle: {"nc": ..., "in_maps": [...]}


def _build_program():
    import concourse.bass as bass
    import concourse.bacc as bacc
    import concourse.mybir as mybir
    import concourse.tile as tile

    nc = bacc.Bacc(trn_type="TRN2", dynamic_dma_scratch_size=65536)
    f32 = mybir.dt.float32
    bf16 = mybir.dt.bfloat16
    i32 = mybir.dt.int32
    d_xs = nc.declare_dram_parameter("xs", [NSH, F], bf16, isOutput=False)
    # All edge/weight metadata rides in one int32 blob (fewer dispatch
    # buffers = lower per-dispatch overhead on this runtime). Layout per
    # partition, in i32 columns:
    #   [0:NCH)        packed gidx: node index bits 0..23, row-local 24..29
    #   [NCH:NCH+500)  vals, 1000 bf16
    #   [+33)          iota 0..65, 66 bf16
    #   [+64)          W row (f32, partitions 0..63)
    #   [+1)           b (f32, partitions 0..63)
    d_blob = nc.declare_dram_parameter("blob", [P, BLOB_COLS], i32, isOutput=False)
    d_out = nc.declare_dram_parameter("out", [BPC, F, W_R], bf16, isOutput=True)

    # Collectives can't touch I/O tensors directly: bounce the local x
    # shard into internal DRAM, AllGather to a full copy per core.
    xs_bounce = nc.dram_tensor("xs_bounce", [NSH, F], bf16)
    xfull = nc.dram_tensor("xfull", [N_NODES, F], bf16, addr_space="Shared")

    with tile.TileContext(nc) as tc:
        nc.gpsimd.dma_start(out=xs_bounce[:], in_=d_xs[:])
        nc.gpsimd.collective_compute(
            "AllGather",
            mybir.AluOpType.bypass,
            replica_groups=[list(range(NCORE))],
            ins=[xs_bounce[:].opt()],
            outs=[xfull[:].opt()],
        )
        with (
            tc.tile_pool(name="const", bufs=1) as constp,
            tc.tile_pool(name="g", bufs=8) as gp,
            tc.tile_pool(name="oh", bufs=8) as ohp,
            tc.tile_pool(name="ev", bufs=4) as evp,
            tc.tile_pool(name="accp", bufs=2, space="PSUM") as accp,
            tc.tile_pool(name="outp", bufs=2, space="PSUM") as outpp,
        ):
            t_blob = constp.tile([P, BLOB_COLS], i32)
            t_gidx = constp.tile([P, NCH], i32)
            t_rli = constp.tile([P, NCH], i32)
            t_rl = constp.tile([P, NCH], f32)
            t_vals = constp.tile([P, NCH], f32)
            nc.sync.dma_start(out=t_blob[:], in_=d_blob[:])
            t_pk = t_blob[:, 0:NCH]
            t_valsb = t_blob[:, NCH : NCH + 500].bitcast(bf16)
            t_iota = t_blob[:, NCH + 500 : NCH + 533].bitcast(bf16)  # [P, 66]
            t_W = t_blob[0:F, NCH + 533 : NCH + 597].bitcast(f32)    # [64, 64]
            t_b = t_blob[0:F, NCH + 597 : NCH + 598].bitcast(f32)    # [64, 1]
            nc.vector.tensor_scalar(
                out=t_rli[:], in0=t_pk, scalar1=24, scalar2=None,
                op0=mybir.AluOpType.logical_shift_right,
            )
            nc.vector.tensor_copy(t_rl[:], t_rli[:])
            nc.vector.tensor_scalar(
                out=t_gidx[:], in0=t_pk, scalar1=0x00FFFFFF, scalar2=None,
                op0=mybir.AluOpType.bitwise_and,
            )
            nc.vector.tensor_copy(t_vals[:], t_valsb)

            for blk in range(BPC):
                t_acc = accp.tile([F, W_R], f32, space="PSUM")
                for ci in range(CPB):
                    c = blk * CPB + ci
                    t_g = gp.tile([P, F], bf16)
                    nc.gpsimd.indirect_dma_start(
                        out=t_g[:],
                        out_offset=None,
                        in_=xfull[:],
                        in_offset=bass.IndirectOffsetOnAxis(
                            ap=t_gidx[:, c : c + 1], axis=0
                        ),
                    )
                    t_oh = ohp.tile([P, 66], bf16)
                    nc.vector.tensor_scalar(
                        out=t_oh[:],
                        in0=t_iota,
                        scalar1=t_rl[:, c : c + 1],
                        scalar2=t_vals[:, c : c + 1],
                        op0=mybir.AluOpType.is_equal,
                        op1=mybir.AluOpType.mult,
                    )
                    nc.tensor.matmul(
                        out=t_acc[:],
                        lhsT=t_g[:],
                        rhs=t_oh[:, :W_R],
                        start=(ci == 0),
                        stop=(ci == CPB - 1),
                    )
                t_accs = evp.tile([F, W_R], f32)
                nc.scalar.copy(t_accs[:], t_acc[:])
                t_out = outpp.tile([F, W_R], f32, space="PSUM")
                nc.tensor.matmul(
                    out=t_out[:], lhsT=t_W, rhs=t_accs[:], start=True, stop=True
                )
                t_outs = evp.tile([F, W_R], bf16)
                nc.scalar.add(t_outs[:], t_out[:], t_b)
                nc.sync.dma_start(out=d_out[blk], in_=t_outs[:])

    nc.finalize()
    return nc


def _pack(rows):
    """LPT bin-packing of nodes into NBLK blocks (<=W_R nodes, <=EPB edges).

    Returns node_block[n], node_local[n]."""
    deg = np.bincount(rows, minlength=N_NODES)
    order = np.argsort(-deg, kind="stable")
    node_block = np.empty(N_NODES, dtype=np.int64)
    node_local = np.empty(N_NODES, dtype=np.int64)
    heap = [(0, b) for b in range(NBLK)]
    heapq.heapify(heap)
    bin_nodes = np.zeros(NBLK, dtype=np.int64)
    bin_edges = np.zeros(NBLK, dtype=np.int64)
    spill = []
    for n in order:
        d = int(deg[n])
        placed = False
        tmp = []
        while heap:
            e, b = heapq.heappop(heap)
            if e != bin_edges[b] or bin_nodes[b] >= W_R:
                continue  # stale or node-full entry
            if e + d <= EPB:
                node_block[n] = b
                node_local[n] = bin_nodes[b]
                bin_nodes[b] += 1
                bin_edges[b] += d
                if bin_nodes[b] < W_R:
                    heapq.heappush(heap, (int(bin_edges[b]), b))
                placed = True
                break
            else:
                tmp.append((e, b))
        for item in tmp:
            heapq.heappush(heap, item)
        if not placed:
            spill.append(n)
    if spill:
        # first-fit for spilled nodes (rare)
        for n in spill:
            d = int(deg[n])
            cand = np.where((bin_nodes < W_R) & (bin_edges + d <= EPB))[0]
            if len(cand) == 0:
                raise RuntimeError("packing failed")
            b = int(cand[0])
            node_block[n] = b
            node_local[n] = bin_nodes[b]
            bin_nodes[b] += 1
            bin_edges[b] += d
    return node_block, node_local


def kernel(x, adj_vals, adj_row, adj_col, W, b):
    rows = np.asarray(adj_row).astype(np.int64)
    cols = np.asarray(adj_col).astype(np.int64)
    vals = np.asarray(adj_vals).astype(np.float32)
    x = np.ascontiguousarray(np.asarray(x, dtype=np.float32))
    W = np.ascontiguousarray(np.asarray(W, dtype=np.float32))
    b = np.asarray(b, dtype=np.float32)

    node_block, node_local = _pack(rows)

    # edge -> (block, slot-within-block)
    eb = node_block[rows]
    order = np.argsort(eb, kind="stable")
    eb_sorted = eb[order]
    counts = np.bincount(eb_sorted, minlength=NBLK)
    starts = np.concatenate([[0], np.cumsum(counts)[:-1]])
    pos = np.arange(N_EDGES) - np.repeat(starts, counts)

    core = eb_sorted // BPC
    chunk = (eb_sorted % BPC) * CPB + pos // P
    part = pos % P

    gidx_all = np.zeros((NCORE, P, NCH), dtype=np.int32)
    vals_all = np.zeros((NCORE, P, NCH), dtype=np.float32)
    packed = (cols[order] | (node_local[rows[order]] << 24)).astype(np.int32)
    gidx_all[core, part, chunk] = packed
    vals_all[core, part, chunk] = vals[order]

    bf16 = ml_dtypes.bfloat16
    x_bf = x.astype(bf16)
    vals_bf = vals_all.astype(bf16)
    iota_np = np.tile(np.arange(66, dtype=np.float32), (P, 1)).astype(bf16)

    # assemble the per-core metadata blob (see _build_program layout)
    blob8_all = np.zeros((NCORE, P, BLOB_COLS * 4), np.uint8)
    for k in range(NCORE):
        bb = blob8_all[k]
        bb[:, 0 : NCH * 4] = gidx_all[k].view(np.uint8).reshape(P, NCH * 4)
        bb[:, NCH * 4 : NCH * 4 + 2000] = vals_bf[k].view(np.uint8).reshape(P, 2000)
        bb[:, 6000:6132] = iota_np.view(np.uint8).reshape(P, 132)
        bb[:F, 6132:6388] = W.view(np.uint8).reshape(F, 256)
        bb[:F, 6388:6392] = b.astype(np.float32).view(np.uint8).reshape(F, 4)
    blob_all = blob8_all.view(np.int32)

    key = "prog"
    if key not in _cache:
        _cache[key] = _build_program()
    nc = _cache[key]

    from concourse.bass_utils import run_bass_kernel_spmd

    in_maps = []
    for k in range(NCORE):
        in_maps.append(
            {
                "xs": np.ascontiguousarray(x_bf[k * NSH : (k + 1) * NSH]),
                "blob": np.ascontiguousarray(blob_all[k]),
            }
        )
    LAST["nc"] = nc
    LAST["in_maps"] = in_maps
    res = run_bass_kernel_spmd(nc, in_maps, list(range(NCORE)))
    LAST["res"] = res

    out_full = np.zeros((N_NODES, F), dtype=np.float32)
    nodes = np.arange(N_NODES)
    nb = node_block[nodes]
    for k in range(NCORE):
        sel = (nb // BPC) == k
        blk = (nb[sel] % BPC).astype(np.int64)
        r = node_local[nodes[sel]].astype(np.int64)
        big = res.results[k]["out"].astype(np.float32)  # [BPC, F, W_R]
        out_full[nodes[sel]] = big[blk, :, r]
    return out_full
